# revision 18
# baseline (speedup 1.0000x reference)
"""Causal linear attention (elu+1 feature map) on 8 Trainium2 NeuronCores.

Full inputs (n=2, l=2048, h=8, d=64) fp32 are sharded over the 16 (n,h)
head-sequences: core i handles pairs (2i, 2i+1). Each core runs a chunked
scan (chunk C=128) over the sequence:

  AT    = Kf @ Qf^T            (per chunk, per pair; causal-masked)
  out   = ATm^T @ Vaug  +  Qf @ S      then  out /= denom
  S    += Kf^T @ Vaug          (PSUM-resident running state, fp32)

Feature map: elu(x)+1 = min(exp(x), max(x+1, 1)).

Key layout trick: Q's transposed features are written into a BLOCK
structure qfT_blocked[(p',d), 256c + 128p + i] nonzero only for p'==p.
One dense kfT lhsT times this blocked rhs yields BOTH pairs' AT in a
single full-width matmul, and blocked lhsT slices give each pair's
inter-chunk term from the (garbage-tolerant) S state. Every matmul has
base-partition-0 operands: PE quadrant (tile_position) matmuls hang
TRN2 when pipelined, so none are used.

S updates are one dense matmul per chunk (lhsT = both pairs' Kf): the
off-pair blocks of S accumulate garbage that no consumer ever reads
(blocked lhsT zeros them out of out2; the block-diagonal columns of the
snapshot are the only ones that matter).

PSUM accumulation banks get their single start=True from a K=1 all-zeros
matmul; every real matmul accumulates (start=False) — order-robust, since
a start=True invalidates its whole 2KB PSUM bank.

Host-side layouts (fp16, all DMAs contiguous):
  qT, kT: (128, 2048)  [(64p + d), (128c + i)]   (host-transposed)
  k,  v : (128, 2048)  [i, 128c + 64p + d]       (natural)
  out   : (128, 2048) fp32, same indexing as k/v.
"""
import numpy as np
from contextlib import ExitStack

import concourse.bacc as bacc
import concourse.bass as bass
import concourse.tile as tile
from concourse import mybir
from concourse.bass_utils import run_bass_kernel_spmd

N, L, H, D = 2, 2048, 8, 64
C = 128                 # chunk length
NCH = L // C            # 16 chunks
GROUP = 8               # chunks per fmap/DMA group
NGRP = NCH // GROUP
PAIRS = 2
W = NCH * PAIRS * D     # 2048
GW = GROUP * PAIRS * D  # 1024 natural cols per group
TW = GROUP * C          # 1024 transposed cols per group
BW = GROUP * PAIRS * C  # 2048 blocked-transposed cols per group
VW = GROUP * PAIRS * (D + 1)   # 1040 v cols per group (with ones col)
SW = PAIRS * (D + 1)    # 130: S cols [S_p0 | ksum_p0 | S_p1 | ksum_p1]

f16 = mybir.dt.float16
f32 = mybir.dt.float32
AF = mybir.ActivationFunctionType
OP = mybir.AluOpType


def build_kernel():
    nc = bacc.Bacc("TRN2", target_bir_lowering=False, debug=False, num_devices=8)
    qT_d = nc.dram_tensor("qT", (C, W), f16, kind="ExternalInput").ap()
    kT_d = nc.dram_tensor("kT", (C, W), f16, kind="ExternalInput").ap()
    k_d = nc.dram_tensor("k", (C, W), f16, kind="ExternalInput").ap()
    v_d = nc.dram_tensor("v", (C, W), f16, kind="ExternalInput").ap()
    o_d = nc.dram_tensor("o", (C, W), f32, kind="ExternalOutput").ap()

    with tile.TileContext(nc) as tc, ExitStack() as ctx:
        consts = ctx.enter_context(tc.tile_pool(name="consts", bufs=1))
        io_pool = ctx.enter_context(tc.tile_pool(name="io", bufs=2))
        fm_pool = ctx.enter_context(tc.tile_pool(name="fm", bufs=2))
        sm_pool = ctx.enter_context(tc.tile_pool(name="sm", bufs=3))
        at_psum = ctx.enter_context(tc.tile_pool(name="at", bufs=3, space="PSUM"))
        out_psum = ctx.enter_context(tc.tile_pool(name="out", bufs=3, space="PSUM"))
        s_psum = ctx.enter_context(tc.tile_pool(name="sp", bufs=1, space="PSUM"))

        zeros = consts.tile([1, 2 * SW], f16)
        nc.vector.memset(zeros, 0.0)

        # maskT[jj, i] = 1 where jj <= i, over 4 (chunk,pair) blocks
        maskT = consts.tile([C, 4 * C], f32)
        m3 = maskT.rearrange("j (b i) -> j b i", b=4)
        nc.gpsimd.memset(maskT, 0.0)
        nc.gpsimd.affine_select(
            out=m3, in_=m3, compare_op=OP.is_gt, fill=1.0,
            base=0, pattern=[[0, 4], [-1, C]], channel_multiplier=1,
        )

        # persistent running state (off-pair blocks accumulate unread garbage)
        S_ps = s_psum.tile([C, SW], f32)
        nc.tensor.matmul(S_ps, zeros[:, 0:C], zeros[:, 0:SW],
                         start=True, stop=False, skip_group_check=True)

        S_sb = None
        for g in range(NGRP):
            gsl = slice(g * GW, (g + 1) * GW)
            tsl = slice(g * TW, (g + 1) * TW)

            qT_g = io_pool.tile([C, TW], f16, tag="qT_g")
            kT_g = io_pool.tile([C, TW], f16, tag="kT_g")
            k_g = io_pool.tile([C, GW], f16, tag="k_g")
            v_g = io_pool.tile([C, VW], f16, tag="v_g")
            nc.sync.dma_start(qT_g, qT_d[:, tsl])
            nc.sync.dma_start(kT_g, kT_d[:, tsl])
            nc.sync.dma_start(k_g, k_d[:, gsl])
            v4 = v_g.rearrange("i (j b x) -> i j b x", j=GROUP, b=PAIRS)
            nc.sync.dma_start(
                v4[:, :, :, 0:D],
                v_d[:, gsl].rearrange("i (j b x) -> i j b x", j=GROUP, b=PAIRS),
            )
            nc.gpsimd.memset(v4[:, :, :, D:D + 1], 1.0)

            # feature maps: f = min(exp(x), max(x+1,1))
            # q: dense exp/clamp, then min writes the pair-diagonal blocks of
            # a zeroed (128, 2048) tile -> qfb
            e_q = fm_pool.tile([C, TW], f16, tag="e_q")
            t_q = fm_pool.tile([C, TW], f16, tag="t_q")
            qfb = fm_pool.tile([C, BW], f16, tag="qfb")
            nc.scalar.activation(e_q, qT_g, AF.Exp)
            nc.gpsimd.tensor_scalar(out=t_q, in0=qT_g, scalar1=1.0, scalar2=1.0,
                                    op0=OP.add, op1=OP.max)
            nc.gpsimd.memset(qfb, 0.0)
            qfb4 = qfb.rearrange("r (j b i) -> r j b i", j=GROUP, b=PAIRS)
            e_q3 = e_q.rearrange("r (j i) -> r j i", j=GROUP)
            t_q3 = t_q.rearrange("r (j i) -> r j i", j=GROUP)
            for p in range(PAIRS):
                rows = slice(p * D, (p + 1) * D)
                nc.vector.tensor_tensor(
                    out=qfb4[rows, :, p, :], in0=e_q3[rows], in1=t_q3[rows],
                    op=OP.min)

            # kT: dense
            e_kT = fm_pool.tile([C, TW], f16, tag="e_kT")
            t_kT = fm_pool.tile([C, TW], f16, tag="t_kT")
            kfT = fm_pool.tile([C, TW], f16, tag="kfT")
            nc.scalar.activation(e_kT, kT_g, AF.Exp)
            nc.gpsimd.tensor_scalar(out=t_kT, in0=kT_g, scalar1=1.0, scalar2=1.0,
                                    op0=OP.add, op1=OP.max)
            nc.vector.tensor_tensor(out=kfT, in0=e_kT, in1=t_kT, op=OP.min)

            # k natural: dense
            e_k = fm_pool.tile([C, GW], f16, tag="e_k")
            t_k = fm_pool.tile([C, GW], f16, tag="t_k")
            kf = fm_pool.tile([C, GW], f16, tag="kf")
            nc.scalar.activation(e_k, k_g, AF.Exp)
            nc.gpsimd.tensor_scalar(out=t_k, in0=k_g, scalar1=1.0, scalar2=1.0,
                                    op0=OP.add, op1=OP.max)
            nc.vector.tensor_tensor(out=kf, in0=e_k, in1=t_k, op=OP.min)

            stage = io_pool.tile([C, GW], f32, tag="stage")

            for jj in range(GROUP // 2):    # two chunks per psum tile
                at_ps = at_psum.tile([C, 4 * C], f32, tag="at")
                out_ps = out_psum.tile([C, 2 * SW], f32, tag="out")
                nc.tensor.matmul(out_ps, zeros[:, 0:C], zeros[:, 0:2 * SW],
                                 start=True, stop=False, skip_group_check=True)

                for dj in range(2):
                    j = 2 * jj + dj
                    c = g * GROUP + j
                    tcs = slice(j * C, (j + 1) * C)

                    # both pairs' AT in one matmul (dense lhsT x blocked rhs)
                    nc.tensor.matmul(
                        at_ps[:, dj * 2 * C:(dj + 1) * 2 * C],
                        kfT[:, tcs], qfb[:, j * 2 * C:(j + 1) * 2 * C],
                        start=True, stop=True)

                    # state snapshot (state after chunk c-1)
                    if c > 0:
                        S_sb = sm_pool.tile([C, SW], f16, tag="s_sb")
                        nc.scalar.copy(S_sb, S_ps)
                        for p in range(PAIRS):
                            vs = slice(p * (D + 1), (p + 1) * (D + 1))
                            nc.tensor.matmul(
                                out_ps[:, dj * SW + vs.start:dj * SW + vs.stop],
                                qfb[:, j * 2 * C + p * C:j * 2 * C + (p + 1) * C],
                                S_sb[:, vs],
                                start=False, stop=False, skip_group_check=True)

                    # state update: one dense matmul (off-pair blocks = junk)
                    nc.tensor.matmul(
                        S_ps,
                        kf[:, j * PAIRS * D:(j + 1) * PAIRS * D],
                        v_g[:, j * SW:(j + 1) * SW],
                        start=False, stop=(c == NCH - 1),
                        skip_group_check=True)

                # mask both chunks' AT at once
                atm = sm_pool.tile([C, 4 * C], f16, tag="atm")
                nc.vector.tensor_mul(atm, at_ps, maskT)

                for dj in range(2):
                    j = 2 * jj + dj
                    c = g * GROUP + j
                    for p in range(PAIRS):
                        vs = slice(dj * SW + p * (D + 1), dj * SW + (p + 1) * (D + 1))
                        nc.tensor.matmul(
                            out_ps[:, vs],
                            atm[:, (2 * dj + p) * C:(2 * dj + p + 1) * C],
                            v4[:, j, p, :],
                            start=False,
                            stop=(dj == 1 and p == PAIRS - 1),
                            skip_group_check=True)

                # out = num * (1/den) for both chunks+pairs
                o5 = out_ps.rearrange("i (a b x) -> i a b x", a=2, b=PAIRS)
                recip = sm_pool.tile([C, 2, PAIRS, 1], f32, tag="recip")
                nc.vector.reciprocal(recip, o5[:, :, :, D:D + 1])
                rec_b = bass.AP(
                    tensor=recip.tensor, offset=recip.offset,
                    ap=[list(recip.ap[0]), list(recip.ap[1]),
                        list(recip.ap[2]), [0, D]],
                )
                st4 = stage.rearrange(
                    "i (j b x) -> i j b x", j=GROUP, b=PAIRS)[:, 2 * jj:2 * jj + 2]
                nc.vector.tensor_tensor(
                    out=st4, in0=o5[:, :, :, 0:D], in1=rec_b, op=OP.mult)

            nc.sync.dma_start(o_d[:, gsl], stage)

    nc.compile()
    return nc


_nc_cache = None


def _get_nc():
    global _nc_cache
    if _nc_cache is None:
        _nc_cache = build_kernel()
    return _nc_cache


def _core_pairs(x, core):
    flat = x.transpose(0, 2, 1, 3).reshape(N * H, L, D)
    return flat[2 * core:2 * core + 2]          # (2, L, D) fp32


def _nat_layout(xc):
    # (2, L, D) -> (128, 2048) [i, 128c + 64p + d]
    return np.ascontiguousarray(
        xc.reshape(PAIRS, NCH, C, D).transpose(2, 1, 0, 3).reshape(C, W)
    ).astype(np.float16)


def _t_layout(xc):
    # (2, L, D) -> (128, 2048) [(64p + d), (128c + i)]
    return np.ascontiguousarray(
        xc.reshape(PAIRS, NCH, C, D).transpose(0, 3, 1, 2).reshape(C, W)
    ).astype(np.float16)


def make_in_maps(queries, keys, values):
    in_maps = []
    for core in range(8):
        qc = _core_pairs(queries, core)
        kc = _core_pairs(keys, core)
        vc = _core_pairs(values, core)
        in_maps.append({
            "qT": _t_layout(qc),
            "kT": _t_layout(kc),
            "k": _nat_layout(kc),
            "v": _nat_layout(vc),
        })
    return in_maps


def kernel(queries, keys, values):
    nc = _get_nc()
    in_maps = make_in_maps(queries, keys, values)
    res = run_bass_kernel_spmd(nc, in_maps, core_ids=list(range(8)))
    out = np.zeros((N, L, H, D), np.float32)
    for core in range(8):
        oc = res.results[core]["o"].reshape(C, NCH, PAIRS, D)
        oc = oc.transpose(2, 1, 0, 3).reshape(PAIRS, L, D)
        for p in range(PAIRS):
            flat = 2 * core + p
            out[flat // H, :, flat % H, :] = oc[p]
    return out


# revision 21
# speedup vs baseline: 2.8992x; 2.8992x over previous
"""Causal linear attention (elu+1 feature map) on 8 Trainium2 NeuronCores.

Full inputs (n=2, l=2048, h=8, d=64) fp32 are sharded over the 16 (n,h)
head-sequences: core i handles pairs (2i, 2i+1). Each core runs a chunked
scan (chunk C=128) over the sequence:

  AT    = Kf @ Qf^T            (per chunk, per pair; causal-masked)
  out   = ATm^T @ Vaug  +  Qf @ S      then  out /= denom
  S    += Kf^T @ Vaug          (PSUM-resident running state, fp32)

Feature map: elu(x)+1 = min(exp(x), max(x+1, 1)).

Key layout trick: Q's transposed features are written into a BLOCK
structure qfT_blocked[(p',d), 256c + 128p + i] nonzero only for p'==p.
One dense kfT lhsT times this blocked rhs yields BOTH pairs' AT in a
single full-width matmul, and blocked lhsT slices give each pair's
inter-chunk term from the (garbage-tolerant) S state. Every matmul has
base-partition-0 operands: PE quadrant (tile_position) matmuls hang
TRN2 when pipelined, so none are used.

S updates are one dense matmul per chunk (lhsT = both pairs' Kf): the
off-pair blocks of S accumulate garbage that no consumer ever reads
(blocked lhsT zeros them out of out2; the block-diagonal columns of the
snapshot are the only ones that matter).

PSUM accumulation banks get their single start=True from a K=1 all-zeros
matmul; every real matmul accumulates (start=False) — order-robust, since
a start=True invalidates its whole 2KB PSUM bank.

Host-side layouts (fp16, all DMAs contiguous):
  qT, kT: (128, 2048)  [(64p + d), (128c + i)]   (host-transposed)
  k,  v : (128, 2048)  [i, 128c + 64p + d]       (natural)
  out   : (128, 2048) fp32, same indexing as k/v.
"""
import numpy as np
from contextlib import ExitStack

import concourse.bacc as bacc
import concourse.bass as bass
import concourse.tile as tile
from concourse import mybir
from concourse.bass_utils import run_bass_kernel_spmd

N, L, H, D = 2, 2048, 8, 64
C = 128                 # chunk length
NCH = L // C            # 16 chunks
GROUP = 8               # chunks per fmap/DMA group
NGRP = NCH // GROUP
PAIRS = 2
W = NCH * PAIRS * D     # 2048
GW = GROUP * PAIRS * D  # 1024 natural cols per group
TW = GROUP * C          # 1024 transposed cols per group
BW = GROUP * PAIRS * C  # 2048 blocked-transposed cols per group
VW = GROUP * PAIRS * (D + 1)   # 1040 v cols per group (with ones col)
SW = PAIRS * (D + 1)    # 130: S cols [S_p0 | ksum_p0 | S_p1 | ksum_p1]

f16 = mybir.dt.float16
f32 = mybir.dt.float32
AF = mybir.ActivationFunctionType
OP = mybir.AluOpType


def build_kernel():
    nc = bacc.Bacc("TRN2", target_bir_lowering=False, debug=False, num_devices=8)
    qT_d = nc.dram_tensor("qT", (C, W), f16, kind="ExternalInput").ap()
    kT_d = nc.dram_tensor("kT", (C, W), f16, kind="ExternalInput").ap()
    k_d = nc.dram_tensor("k", (C, W), f16, kind="ExternalInput").ap()
    v_d = nc.dram_tensor("v", (C, W), f16, kind="ExternalInput").ap()
    o_d = nc.dram_tensor("o", (C, W), f32, kind="ExternalOutput").ap()

    with tile.TileContext(nc) as tc, ExitStack() as ctx:
        consts = ctx.enter_context(tc.tile_pool(name="consts", bufs=1))
        io_pool = ctx.enter_context(tc.tile_pool(name="io", bufs=2))
        fm_pool = ctx.enter_context(tc.tile_pool(name="fm", bufs=2))
        sm_pool = ctx.enter_context(tc.tile_pool(name="sm", bufs=3))
        at_psum = ctx.enter_context(tc.tile_pool(name="at", bufs=3, space="PSUM"))
        out_psum = ctx.enter_context(tc.tile_pool(name="out", bufs=3, space="PSUM"))
        s_psum = ctx.enter_context(tc.tile_pool(name="sp", bufs=1, space="PSUM"))

        zeros = consts.tile([1, 2 * SW], f16)
        nc.vector.memset(zeros, 0.0)

        # maskT[jj, i] = 1 where jj <= i, over 4 (chunk,pair) blocks
        maskT = consts.tile([C, 4 * C], f32)
        m3 = maskT.rearrange("j (b i) -> j b i", b=4)
        nc.gpsimd.memset(maskT, 0.0)
        nc.gpsimd.affine_select(
            out=m3, in_=m3, compare_op=OP.is_gt, fill=1.0,
            base=0, pattern=[[0, 4], [-1, C]], channel_multiplier=1,
        )

        # persistent running state (off-pair blocks accumulate unread garbage)
        S_ps = s_psum.tile([C, SW], f32)
        nc.tensor.matmul(S_ps, zeros[:, 0:C], zeros[:, 0:SW],
                         start=True, stop=False, skip_group_check=True)

        S_sb = None
        for g in range(NGRP):
            gsl = slice(g * GW, (g + 1) * GW)
            tsl = slice(g * TW, (g + 1) * TW)

            qT_g = io_pool.tile([C, TW], f16, tag="qT_g")
            kT_g = io_pool.tile([C, TW], f16, tag="kT_g")
            k_g = io_pool.tile([C, GW], f16, tag="k_g")
            v_g = io_pool.tile([C, VW], f16, tag="v_g")
            nc.sync.dma_start(qT_g, qT_d[:, tsl])
            nc.sync.dma_start(kT_g, kT_d[:, tsl])
            nc.sync.dma_start(k_g, k_d[:, gsl])
            v4 = v_g.rearrange("i (j b x) -> i j b x", j=GROUP, b=PAIRS)
            nc.sync.dma_start(
                v4[:, :, :, 0:D],
                v_d[:, gsl].rearrange("i (j b x) -> i j b x", j=GROUP, b=PAIRS),
            )
            nc.vector.memset(v4[:, :, :, D:D + 1], 1.0)

            # feature maps: f = min(exp(x), max(x+1,1))
            # q: dense exp/clamp, then min writes the pair-diagonal blocks of
            # a zeroed tile -> qfb, pair-major: col = 1024p + 128j + i, so the
            # min per pair is a dense 2D (64 x 1024) op.
            e_q = fm_pool.tile([C, TW], f16, tag="e_q")
            t_q = fm_pool.tile([C, TW], f16, tag="t_q")
            qfb = fm_pool.tile([C, BW], f16, tag="qfb")
            nc.scalar.activation(e_q, qT_g, AF.Exp)
            nc.vector.tensor_scalar(out=t_q, in0=qT_g, scalar1=1.0, scalar2=1.0,
                                    op0=OP.add, op1=OP.max)
            nc.vector.memset(qfb, 0.0)
            for p in range(PAIRS):
                rows = slice(p * D, (p + 1) * D)
                nc.vector.tensor_tensor(
                    out=qfb[rows, p * TW:(p + 1) * TW],
                    in0=e_q[rows], in1=t_q[rows], op=OP.min)

            # kT: dense
            e_kT = fm_pool.tile([C, TW], f16, tag="e_kT")
            t_kT = fm_pool.tile([C, TW], f16, tag="t_kT")
            kfT = fm_pool.tile([C, TW], f16, tag="kfT")
            nc.scalar.activation(e_kT, kT_g, AF.Exp)
            nc.vector.tensor_scalar(out=t_kT, in0=kT_g, scalar1=1.0, scalar2=1.0,
                                    op0=OP.add, op1=OP.max)
            nc.vector.tensor_tensor(out=kfT, in0=e_kT, in1=t_kT, op=OP.min)

            # k natural: dense
            e_k = fm_pool.tile([C, GW], f16, tag="e_k")
            t_k = fm_pool.tile([C, GW], f16, tag="t_k")
            kf = fm_pool.tile([C, GW], f16, tag="kf")
            nc.scalar.activation(e_k, k_g, AF.Exp)
            nc.vector.tensor_scalar(out=t_k, in0=k_g, scalar1=1.0, scalar2=1.0,
                                    op0=OP.add, op1=OP.max)
            nc.vector.tensor_tensor(out=kf, in0=e_k, in1=t_k, op=OP.min)

            stage = io_pool.tile([C, GW], f32, tag="stage")

            for jj in range(GROUP // 2):    # two chunks per psum tile
                at_ps = at_psum.tile([C, 4 * C], f32, tag="at")
                out_ps = out_psum.tile([C, 2 * SW], f32, tag="out")
                nc.tensor.matmul(out_ps, zeros[:, 0:C], zeros[:, 0:2 * SW],
                                 start=True, stop=False, skip_group_check=True)

                for dj in range(2):
                    j = 2 * jj + dj
                    c = g * GROUP + j
                    tcs = slice(j * C, (j + 1) * C)

                    # both pairs' AT in one matmul (dense lhsT x blocked rhs)
                    qfb3 = qfb.rearrange("r (p x) -> r p x", p=PAIRS)
                    nc.tensor.matmul(
                        at_ps[:, dj * 2 * C:(dj + 1) * 2 * C],
                        kfT[:, tcs], qfb3[:, :, j * C:(j + 1) * C],
                        start=True, stop=True)

                    # state snapshot (state after chunk c-1)
                    if c > 0:
                        S_sb = sm_pool.tile([C, SW], f16, tag="s_sb")
                        nc.scalar.copy(S_sb, S_ps)
                        for p in range(PAIRS):
                            vs = slice(p * (D + 1), (p + 1) * (D + 1))
                            nc.tensor.matmul(
                                out_ps[:, dj * SW + vs.start:dj * SW + vs.stop],
                                qfb[:, p * TW + j * C:p * TW + (j + 1) * C],
                                S_sb[:, vs],
                                start=False, stop=False, skip_group_check=True)

                    # state update: one dense matmul (off-pair blocks = junk)
                    nc.tensor.matmul(
                        S_ps,
                        kf[:, j * PAIRS * D:(j + 1) * PAIRS * D],
                        v_g[:, j * SW:(j + 1) * SW],
                        start=False, stop=(c == NCH - 1),
                        skip_group_check=True)

                # mask both chunks' AT at once
                atm = sm_pool.tile([C, 4 * C], f16, tag="atm")
                nc.vector.tensor_mul(atm, at_ps, maskT)

                for dj in range(2):
                    j = 2 * jj + dj
                    c = g * GROUP + j
                    for p in range(PAIRS):
                        vs = slice(dj * SW + p * (D + 1), dj * SW + (p + 1) * (D + 1))
                        nc.tensor.matmul(
                            out_ps[:, vs],
                            atm[:, (2 * dj + p) * C:(2 * dj + p + 1) * C],
                            v4[:, j, p, :],
                            start=False,
                            stop=(dj == 1 and p == PAIRS - 1),
                            skip_group_check=True)

                # out = num * (1/den) for both chunks+pairs
                o5 = out_ps.rearrange("i (a b x) -> i a b x", a=2, b=PAIRS)
                recip = sm_pool.tile([C, 2, PAIRS, 1], f32, tag="recip")
                nc.vector.reciprocal(recip, o5[:, :, :, D:D + 1])
                rec_b = bass.AP(
                    tensor=recip.tensor, offset=recip.offset,
                    ap=[list(recip.ap[0]), list(recip.ap[1]),
                        list(recip.ap[2]), [0, D]],
                )
                st4 = stage.rearrange(
                    "i (j b x) -> i j b x", j=GROUP, b=PAIRS)[:, 2 * jj:2 * jj + 2]
                nc.vector.tensor_tensor(
                    out=st4, in0=o5[:, :, :, 0:D], in1=rec_b, op=OP.mult)

            nc.sync.dma_start(o_d[:, gsl], stage)

    nc.compile()
    return nc


_nc_cache = None


def _get_nc():
    global _nc_cache
    if _nc_cache is None:
        _nc_cache = build_kernel()
    return _nc_cache


def _core_pairs(x, core):
    flat = x.transpose(0, 2, 1, 3).reshape(N * H, L, D)
    return flat[2 * core:2 * core + 2]          # (2, L, D) fp32


def _nat_layout(xc):
    # (2, L, D) -> (128, 2048) [i, 128c + 64p + d]
    return np.ascontiguousarray(
        xc.reshape(PAIRS, NCH, C, D).transpose(2, 1, 0, 3).reshape(C, W)
    ).astype(np.float16)


def _t_layout(xc):
    # (2, L, D) -> (128, 2048) [(64p + d), (128c + i)]
    return np.ascontiguousarray(
        xc.reshape(PAIRS, NCH, C, D).transpose(0, 3, 1, 2).reshape(C, W)
    ).astype(np.float16)


def make_in_maps(queries, keys, values):
    in_maps = []
    for core in range(8):
        qc = _core_pairs(queries, core)
        kc = _core_pairs(keys, core)
        vc = _core_pairs(values, core)
        in_maps.append({
            "qT": _t_layout(qc),
            "kT": _t_layout(kc),
            "k": _nat_layout(kc),
            "v": _nat_layout(vc),
        })
    return in_maps


def kernel(queries, keys, values):
    nc = _get_nc()
    in_maps = make_in_maps(queries, keys, values)
    res = run_bass_kernel_spmd(nc, in_maps, core_ids=list(range(8)))
    out = np.zeros((N, L, H, D), np.float32)
    for core in range(8):
        oc = res.results[core]["o"].reshape(C, NCH, PAIRS, D)
        oc = oc.transpose(2, 1, 0, 3).reshape(PAIRS, L, D)
        for p in range(PAIRS):
            flat = 2 * core + p
            out[flat // H, :, flat % H, :] = oc[p]
    return out
